# revision 26
# baseline (speedup 1.0000x reference)
"""Trainium2 Bass kernel for nn_Conv2DLinalgRMSNorm (two launches, bf16).

Math: out = RMSNormEps(x @ (sum_l conv_w[l])^T / 20) * norm_w
  where RMSNormEps(v) = v / sqrt(sum_h v^2 + eps*H) * sqrt(H)
The 1/20 folds into the norm bias: with y = x @ Wsum^T,
  out = y * sqrt(H) / sqrt(sum y^2 + NL^2*eps*H) * norm_w.

Strategy (8 NeuronCores):
  Host does dtype conversion / transposition / piece assembly only
  (all arithmetic stays on device).
  Launch 1 (weight prep): core c owns output-channel rows [128c,128c+128)
    of the 20 conv weights, laid out [8 hc][128 h][20 l][128 o] bf16.
    Per chunk a 5-level pairwise bf16 add tree on DVE (2x_1P mode) sums
    the 20 layers into piece[:, hc, :]. Reduction runs ONLY on DVE:
    GpSimd shares DVE's SBUF port (exclusive lock) so splitting across
    both self-contends, and tensor_reduce only runs in 1x mode.
    DMA: one chunk per transfer on a single ring (sync), chunk-ordered so
    DVE starts on chunk 0 early; one piece write at the end on the same
    ring. Only 2 engines are used, which keeps the semaphore count (and
    the per-semaphore epilogue-reset stream) small.
  Launch 2 (token-parallel GEMM + norm): core c takes 1024 tokens.
    Weights/tokens arrive as 16 SEPARATE SBUF tiles (8 wt chunks on the
    sync ring, 8 xh half-groups on the scalar ring) so Tile's whole-tile
    dependency tracking lets matmuls fire per chunk instead of after the
    full weight matrix. 10 PE warm-up matmuls (on a DVE-memset tile)
    bridge the DMA-ring cold-start ramp so HAM is at 2.4 GHz and data is
    resident when real matmuls start. Tiles 0-2 run hc-outer (consume
    chunks as they arrive); tiles 3-7 run tile-outer so PSUM-bank frees
    lead reuse and the staggered stops pipeline the norm chains.
    Norm per tile keeps the DVE nearly free and uses one ACT table set:
      ACT Square(pt)+accum: vb = sum(y^2) read straight from PSUM
      ACT Copy pt->yc bf16 (frees the PSUM banks)
      DVE +eps-bias, reciprocal; ACT Sqrt(H * rv)
      DVE tensor_scalar_mul yt = yc*s (4x), tensor_tensor ysb = yt*nw (2x)
    y written bf16 on the sync ring; scratch writes at tiles 3/5 re-warm
    the idle ring before the y stream. Output upcast on host.
"""
import numpy as np
import ml_dtypes

import concourse.bass as bass
import concourse.mybir as mybir
from concourse.tile import TileContext
from concourse import bass_utils

dt = mybir.dt
P = 128
H = 1024
NL = 20
B, S = 2, 4096
TOK = B * S            # 8192
NCORES = 8
TPC = TOK // NCORES    # 1024 tokens per core
NCH = 8                # h chunks of 128
NTG = 4                # token groups of 256
TGW = TPC // NTG       # 256
NT = TPC // P          # 8 token tiles
EPS = 1e-6
SSQ_BIAS = float(NL * NL * EPS * H)   # 0.4096
NHALF = 2          # L1 chunks pre-split into o-halves for early DVE start

_ctr = [0]


def _legalize_waits(nc):
    """This walrus build accepts 1 sync wait per instruction (2 on
    EventSemaphore); split excess waits into standalone waits."""
    def fix_block(blk):
        insts = list(blk.instructions)
        out = []
        changed = False
        for inst in insts:
            si = inst.sync_info
            waits = list(si.on_wait) if si and si.on_wait else []
            cap = 2 if isinstance(inst, mybir.InstEventSemaphore) else 1
            if len(waits) > cap:
                changed = True
                keep = waits[:cap]
                extra = waits[cap:]
                for i in range(0, len(extra), 2):
                    chunk = extra[i:i + 2]
                    _ctr[0] += 1
                    ev = mybir.InstEventSemaphore(
                        name=f"I-waitfix-{_ctr[0]}",
                        engine=inst.engine,
                        ins=[],
                        outs=[],
                        sync_info=mybir.SyncInfo(on_wait=chunk, on_update=[]),
                    )
                    out.append(ev)
                si.on_wait = keep
            out.append(inst)
        if changed:
            blk.instructions = out
        for sub in getattr(blk, "blocks", None) or []:
            fix_block(sub)

    for fn in nc.m.functions:
        for blk in fn.blocks:
            fix_block(blk)


def build_wprep():
    """Launch 1: cw [8 hc,128 h,20 l,128 o] bf16 -> piece [128,8,128] bf16."""
    nc = bass.Bass('TRN2', target_bir_lowering=False, debug=False)
    cw = nc.dram_tensor("cw", [NCH, P, NL, P], dt.bfloat16, kind="ExternalInput")
    piece = nc.dram_tensor("piece", [P, NCH, P], dt.bfloat16, kind="ExternalOutput")
    with TileContext(nc) as tc:
        with (
            tc.tile_pool(name="cwp", bufs=NCH) as cwp,
            tc.tile_pool(name="up", bufs=2) as up,
            tc.tile_pool(name="pc", bufs=1) as pcp,
        ):
            pall = pcp.tile([P, NCH, P], dt.bfloat16, tag="pall")
            # chunk 0 split into two SEPARATE l-half tiles so the first add
            # depends only on the first half (whole-tile dep tracking)
            t0a = cwp.tile([P, NL // 2, P], dt.bfloat16, tag="cw0a")
            nc.sync.dma_start(t0a[:], cw[0, :, 0:NL // 2])
            t0b = cwp.tile([P, NL // 2, P], dt.bfloat16, tag="cw0b")
            nc.sync.dma_start(t0b[:], cw[0, :, NL // 2:NL])
            tiles = [(t0a, t0b)]
            for hc in range(1, NCH):
                t = cwp.tile([P, NL, P], dt.bfloat16, tag="cw", name=f"cw{hc}")
                nc.sync.dma_start(t[:], cw[hc])
                tiles.append(t)
            # levels 1-2 per chunk; levels 3-5 merged across chunk pairs
            # (wider DVE ops amortize the ~150ns per-instruction overhead)
            for k in range(NCH // 2):
                u2p = up.tile([P, 2, 5, P], dt.bfloat16, tag="u2p",
                              name=f"u2p{k}")
                for j in range(2):
                    hc = 2 * k + j
                    t = tiles[hc]
                    if hc == 0:
                        a = up.tile([P, 5, P], dt.bfloat16, tag="a0", name="a0")
                        nc.vector.tensor_add(a[:], t[0][:, 0:5], t[0][:, 5:10])
                        b = up.tile([P, 5, P], dt.bfloat16, tag="b0", name="b0")
                        nc.vector.tensor_add(b[:], t[1][:, 0:5], t[1][:, 5:10])
                        nc.vector.tensor_add(u2p[:, j], a[:], b[:])
                        continue
                    u1 = up.tile([P, 10, P], dt.bfloat16, tag="u1",
                                 name=f"u1_{hc}")
                    nc.vector.tensor_add(u1[:], t[:, 0:10], t[:, 10:20])
                    nc.vector.tensor_add(u2p[:, j], u1[:, 0:5], u1[:, 5:10])
                u3 = up.tile([P, 2, 2, P], dt.bfloat16, tag="u3p",
                             name=f"u3p{k}")
                nc.vector.tensor_add(u3[:], u2p[:, :, 0:2], u2p[:, :, 2:4])
                u4 = up.tile([P, 2, P], dt.bfloat16, tag="u4p", name=f"u4p{k}")
                nc.vector.tensor_add(u4[:], u3[:, :, 0], u3[:, :, 1])
                nc.vector.tensor_add(pall[:, 2 * k:2 * k + 2, :], u4[:],
                                     u2p[:, :, 4])
                nc.scalar.dma_start(piece[:, 2 * k:2 * k + 2, :],
                                    pall[:, 2 * k:2 * k + 2, :])
    _legalize_waits(nc)
    return nc


def build_gemm():
    """Launch 2: xh [4,128,8,256] bf16 @ wt [128,8,8,128] bf16 + RMSNorm."""
    nc = bass.Bass('TRN2', target_bir_lowering=False, debug=False)
    xh = nc.dram_tensor("xh", [NTG, P, NCH, TGW], dt.bfloat16, kind="ExternalInput")
    wt = nc.dram_tensor("wt", [P, NCH, NCORES, P], dt.bfloat16, kind="ExternalInput")
    nw = nc.dram_tensor("nw", [H], dt.bfloat16, kind="ExternalInput")
    y = nc.dram_tensor("y", [TPC, H], dt.bfloat16, kind="ExternalOutput")
    scratch = nc.dram_tensor("scratch", [P, H], dt.bfloat16, kind="Internal")
    mult = mybir.AluOpType.mult
    with TileContext(nc) as tc:
        with (
            tc.tile_pool(name="w", bufs=1) as wp,
            tc.tile_pool(name="sq", bufs=2) as sqp,
            tc.tile_pool(name="stat", bufs=8) as stat,
            tc.tile_pool(name="y", bufs=3) as yp,
            tc.tile_pool(name="psum", bufs=4, space="PSUM") as psum,
        ):
            nwb = wp.tile([P, H], dt.bfloat16, tag="nwb")
            wm = wp.tile([P, 512], dt.bfloat16, tag="wm")

            # PE warm-up while the first chunks stream in (cold PE = 1.2 GHz)
            nc.vector.memset(wm[:], 0.0)
            wu = psum.tile([P, H], dt.float32, tag="pt", name="wu")
            for i in range(12):
                nc.tensor.matmul(wu[:, 0:512], wm[:, 0:P], wm[:],
                                 start=True, stop=True)

            # separate SBUF tiles per chunk so matmuls depend only on the
            # chunk they read; single input ring (sync), priority order.
            # hc=0 is extra-fine (wt o-halves, per-tg hc0 x slices) so the
            # first matmuls fire while the DMA ring is still ramping up.
            wtc = [wp.tile([P, NCORES, P], dt.bfloat16, tag=f"wt{hc}",
                           name=f"wtc{hc}") for hc in range(NCH)]
            xhh = {(tg, hf): wp.tile([P, NCH // 2, TGW], dt.bfloat16,
                                     tag=f"xh{tg}_{hf}", name=f"xhh{tg}_{hf}")
                   for tg in range(NTG) for hf in range(2)}

            def dma_wt(hc):
                nc.sync.dma_start(wtc[hc][:], wt[:, hc])

            def dma_xh(tg, hf):
                nc.scalar.dma_start(xhh[tg, hf][:],
                                  xh[tg, :, hf * (NCH // 2):(hf + 1) * (NCH // 2)])

            dma_wt(0)
            dma_xh(0, 0)
            dma_xh(1, 0)
            dma_wt(1)
            dma_wt(2)
            dma_xh(0, 1)
            dma_xh(1, 1)
            dma_wt(3)
            dma_wt(4)
            dma_xh(2, 0)
            dma_xh(3, 0)
            dma_wt(5)
            dma_wt(6)
            dma_xh(2, 1)
            dma_xh(3, 1)
            dma_wt(7)
            nc.scalar.dma_start(nwb[:], nw[None, :].partition_broadcast(P))

            def norm_tile(tt, pt):
                # ssq on ACT (Square+accum, reads PSUM; Square/Sqrt share one
                # table set), then ACT Copy evicts PSUM to bf16; DVE only does
                # the tiny stats plus the 4x/2x scale ops
                sq = sqp.tile([P, H], dt.bfloat16, tag="sq", name=f"sq{tt}")
                vb = stat.tile([P, 1], dt.float32, tag="vb", name=f"vb{tt}")
                nc.scalar.activation(sq[:], pt[:],
                                     mybir.ActivationFunctionType.Square,
                                     accum_out=vb[:])
                yc = yp.tile([P, H], dt.bfloat16, tag="yc", name=f"yc{tt}")
                if tt == NT - 1:
                    # last tile: evict on DVE, concurrent with ACT's Square
                    # (nothing queues behind it on the DVE FIFO)
                    nc.vector.tensor_copy(yc[:], pt[:])
                else:
                    nc.scalar.activation(yc[:], pt[:],
                                         mybir.ActivationFunctionType.Copy)
                nc.vector.tensor_scalar(
                    vb[:], vb[:], SSQ_BIAS, None, mybir.AluOpType.add,
                )
                rv = stat.tile([P, 1], dt.float32, tag="rv", name=f"rv{tt}")
                nc.vector.reciprocal(rv[:], vb[:])
                s = stat.tile([P, 1], dt.float32, tag="s", name=f"s{tt}")
                nc.scalar.activation(
                    s[:], rv[:], mybir.ActivationFunctionType.Sqrt,
                    scale=float(H),
                )
                yt = yp.tile([P, H], dt.bfloat16, tag="yt", name=f"yt{tt}")
                nc.vector.tensor_scalar_mul(yt[:], yc[:], s[:])
                ysb = yp.tile([P, H], dt.bfloat16, tag="ysb", name=f"ysb{tt}")
                nc.vector.tensor_tensor(ysb[:], yt[:], nwb[:], mult)
                if tt in (3, 5, 6):
                    # contiguous dummy writes re-warm the idle output ring
                    # before/while the y stream runs
                    half = (tt - 3) % 2
                    nc.sync.dma_start(scratch[:, half * 512:(half + 1) * 512],
                                      yc[:, half * 512:(half + 1) * 512])
                nc.sync.dma_start(y[tt * P:(tt + 1) * P, :], ysb[:])

            def mm_tile(pt, tt, hc):
                tg, th = tt // 2, (tt % 2) * P
                lhsT = xhh[tg, hc // 4][:, hc % 4, th:th + P]
                for oh in range(2):
                    nc.tensor.matmul(
                        pt[:, oh * 512:(oh + 1) * 512],
                        lhsT,
                        wtc[hc][:, 4 * oh:4 * oh + 4, :],
                        start=(hc == 0), stop=(hc == NCH - 1),
                    )

            # group 0 (tiles 0-2): hc-outer — start on the first chunk
            pts0 = [psum.tile([P, H], dt.float32, tag="pt", name=f"pt{tt}")
                    for tt in range(3)]
            for hc in range(NCH):
                for tt in range(3):
                    mm_tile(pts0[tt], tt, hc)
            for tt in range(3):
                norm_tile(tt, pts0[tt])

            # group 1 (tiles 3-7): tile-outer — weights resident, staggered
            # stops; PSUM banks freed by group-0 norms well before reuse
            for tt in range(3, NT):
                pt = psum.tile([P, H], dt.float32, tag="pt", name=f"pt{tt}")
                for hc in range(NCH):
                    mm_tile(pt, tt, hc)
                norm_tile(tt, pt)
    _legalize_waits(nc)
    return nc


_CACHE = {}


def _get(name, builder):
    if name not in _CACHE:
        _CACHE[name] = builder()
    return _CACHE[name]


def make_wprep_inputs(conv_w):
    """[20,1024,1024] f32 -> per-core [8 hc,128 h,20 l,128 o] bf16."""
    bf16 = ml_dtypes.bfloat16
    conv_w = np.asarray(conv_w, dtype=np.float32)
    in_maps = []
    for c in range(NCORES):
        a = conv_w[:, c * P:(c + 1) * P, :]          # [20 l, 128 o, 1024 h]
        r = a.reshape(NL, P, NCH, P)                 # [l, o, hc, h]
        cwc = r.transpose(2, 3, 0, 1)                # [hc, h, l, o]
        in_maps.append({"cw": np.ascontiguousarray(cwc.astype(bf16))})
    return in_maps


def assemble_wt(pieces):
    """8 x [128 h,8 hc,128 o_c] -> [128 p(h),8 hc,8 c,128 o] bf16."""
    stacked = np.stack(pieces, axis=2)               # [h, hc, c, o]
    return np.ascontiguousarray(stacked.astype(ml_dtypes.bfloat16))


def make_gemm_inputs(hidden_states, wt_host, norm_w):
    bf16 = ml_dtypes.bfloat16
    x = np.asarray(hidden_states, dtype=np.float32).reshape(TOK, H)
    nw = np.ascontiguousarray(np.asarray(norm_w, dtype=np.float32).astype(bf16))
    in_maps = []
    for c in range(NCORES):
        xc = x[c * TPC:(c + 1) * TPC]                # [1024 t, 1024 h]
        xhc = np.ascontiguousarray(
            xc.reshape(NTG, TGW, NCH, P).transpose(0, 3, 2, 1).astype(bf16)
        )
        in_maps.append({"xh": xhc, "wt": wt_host, "nw": nw})
    return in_maps


def kernel(hidden_states, conv_w, norm_w):
    in_dtype = np.asarray(hidden_states).dtype
    core_ids = list(range(NCORES))

    nc1 = _get("wprep", build_wprep)
    res1 = bass_utils.run_bass_kernel_spmd(nc1, make_wprep_inputs(conv_w), core_ids)
    wt_host = assemble_wt([res1.results[i]["piece"] for i in range(NCORES)])

    nc2 = _get("gemm", build_gemm)
    res2 = bass_utils.run_bass_kernel_spmd(
        nc2, make_gemm_inputs(hidden_states, wt_host, norm_w), core_ids)
    ys = [res2.results[i]["y"].astype(np.float32) for i in range(NCORES)]
    return np.concatenate(ys, axis=0).reshape(B, S, H).astype(in_dtype, copy=False)


# revision 28
# speedup vs baseline: 1.0399x; 1.0399x over previous
"""Trainium2 Bass kernel for nn_Conv2DLinalgRMSNorm (two launches, bf16).

Math: out = RMSNormEps(x @ (sum_l conv_w[l])^T / 20) * norm_w
  where RMSNormEps(v) = v / sqrt(sum_h v^2 + eps*H) * sqrt(H)
The 1/20 folds into the norm bias: with y = x @ Wsum^T,
  out = y * sqrt(H) / sqrt(sum y^2 + NL^2*eps*H) * norm_w.

Strategy (8 NeuronCores):
  Host does dtype conversion / transposition / piece assembly only
  (all arithmetic stays on device).
  Launch 1 (weight prep): core c owns output-channel rows [128c,128c+128)
    of the 20 conv weights, laid out [8 hc][128 h][20 l][128 o] bf16.
    Per chunk a 5-level pairwise bf16 add tree on DVE (2x_1P mode) sums
    the 20 layers into piece[:, hc, :]. Reduction runs ONLY on DVE:
    GpSimd shares DVE's SBUF port (exclusive lock) so splitting across
    both self-contends, and tensor_reduce only runs in 1x mode.
    DMA: one chunk per transfer on a single ring (sync), chunk-ordered so
    DVE starts on chunk 0 early; one piece write at the end on the same
    ring. Only 2 engines are used, which keeps the semaphore count (and
    the per-semaphore epilogue-reset stream) small.
  Launch 2 (token-parallel GEMM + norm): core c takes 1024 tokens.
    Weights/tokens arrive as 16 SEPARATE SBUF tiles (8 wt chunks on the
    sync ring, 8 xh half-groups on the scalar ring) so Tile's whole-tile
    dependency tracking lets matmuls fire per chunk instead of after the
    full weight matrix. 10 PE warm-up matmuls (on a DVE-memset tile)
    bridge the DMA-ring cold-start ramp so HAM is at 2.4 GHz and data is
    resident when real matmuls start. Tiles 0-2 run hc-outer (consume
    chunks as they arrive); tiles 3-7 run tile-outer so PSUM-bank frees
    lead reuse and the staggered stops pipeline the norm chains.
    Norm per tile keeps the DVE nearly free and uses one ACT table set:
      ACT Square(pt)+accum: vb = sum(y^2) read straight from PSUM
      ACT Copy pt->yc bf16 (frees the PSUM banks)
      DVE +eps-bias, reciprocal; ACT Sqrt(H * rv)
      DVE tensor_scalar_mul yt = yc*s (4x), tensor_tensor ysb = yt*nw (2x)
    y written bf16 on the sync ring; scratch writes at tiles 3/5 re-warm
    the idle ring before the y stream. Output upcast on host.
"""
import numpy as np
import ml_dtypes

import concourse.bass as bass
import concourse.mybir as mybir
from concourse.tile import TileContext
from concourse import bass_utils

dt = mybir.dt
P = 128
H = 1024
NL = 20
B, S = 2, 4096
TOK = B * S            # 8192
NCORES = 8
TPC = TOK // NCORES    # 1024 tokens per core
NCH = 8                # h chunks of 128
NTG = 4                # token groups of 256
TGW = TPC // NTG       # 256
NT = TPC // P          # 8 token tiles
EPS = 1e-6
SSQ_BIAS = float(NL * NL * EPS * H)   # 0.4096
NHALF = 2          # L1 chunks pre-split into o-halves for early DVE start

_ctr = [0]


def _legalize_waits(nc):
    """This walrus build accepts 1 sync wait per instruction (2 on
    EventSemaphore); split excess waits into standalone waits."""
    def fix_block(blk):
        insts = list(blk.instructions)
        out = []
        changed = False
        for inst in insts:
            si = inst.sync_info
            waits = list(si.on_wait) if si and si.on_wait else []
            cap = 2 if isinstance(inst, mybir.InstEventSemaphore) else 1
            if len(waits) > cap:
                changed = True
                keep = waits[:cap]
                extra = waits[cap:]
                for i in range(0, len(extra), 2):
                    chunk = extra[i:i + 2]
                    _ctr[0] += 1
                    ev = mybir.InstEventSemaphore(
                        name=f"I-waitfix-{_ctr[0]}",
                        engine=inst.engine,
                        ins=[],
                        outs=[],
                        sync_info=mybir.SyncInfo(on_wait=chunk, on_update=[]),
                    )
                    out.append(ev)
                si.on_wait = keep
            out.append(inst)
        if changed:
            blk.instructions = out
        for sub in getattr(blk, "blocks", None) or []:
            fix_block(sub)

    for fn in nc.m.functions:
        for blk in fn.blocks:
            fix_block(blk)


def build_wprep():
    """Launch 1: cw [8 hc,128 h,20 l,128 o] bf16 -> piece [128,8,128] bf16."""
    nc = bass.Bass('TRN2', target_bir_lowering=False, debug=False)
    cw = nc.dram_tensor("cw", [NCH, P, NL, P], dt.bfloat16, kind="ExternalInput")
    piece = nc.dram_tensor("piece", [P, NCH, P], dt.bfloat16, kind="ExternalOutput")
    with TileContext(nc) as tc:
        with (
            tc.tile_pool(name="cwp", bufs=NCH) as cwp,
            tc.tile_pool(name="up", bufs=2) as up,
            tc.tile_pool(name="pc", bufs=1) as pcp,
        ):
            pall = pcp.tile([P, NCH, P], dt.bfloat16, tag="pall")
            tiles = []
            for hc in range(NCH):
                t = cwp.tile([P, NL, P], dt.bfloat16, tag="cw", name=f"cw{hc}")
                nc.sync.dma_start(t[:], cw[hc])
                tiles.append(t)
            # levels 1-2 per chunk; levels 3-5 merged across chunk pairs
            # (wider DVE ops amortize the ~150ns per-instruction overhead)
            for k in range(NCH // 2 - 1):
                u2p = up.tile([P, 2, 5, P], dt.bfloat16, tag="u2p",
                              name=f"u2p{k}")
                for j in range(2):
                    hc = 2 * k + j
                    t = tiles[hc]
                    u1 = up.tile([P, 10, P], dt.bfloat16, tag="u1",
                                 name=f"u1_{hc}")
                    nc.vector.tensor_add(u1[:], t[:, 0:10], t[:, 10:20])
                    nc.vector.tensor_add(u2p[:, j], u1[:, 0:5], u1[:, 5:10])
                u3 = up.tile([P, 2, 2, P], dt.bfloat16, tag="u3p",
                             name=f"u3p{k}")
                nc.vector.tensor_add(u3[:], u2p[:, :, 0:2], u2p[:, :, 2:4])
                u4 = up.tile([P, 2, P], dt.bfloat16, tag="u4p", name=f"u4p{k}")
                nc.vector.tensor_add(u4[:], u3[:, :, 0], u3[:, :, 1])
                nc.vector.tensor_add(pall[:, 2 * k:2 * k + 2, :], u4[:],
                                     u2p[:, :, 4])
                nc.scalar.dma_start(piece[:, 2 * k:2 * k + 2, :],
                                    pall[:, 2 * k:2 * k + 2, :])
            # last two chunks as singles: less add-work serialized behind
            # the final chunk's DMA (the launch's critical tail)
            for hc in (NCH - 2, NCH - 1):
                t = tiles[hc]
                u1 = up.tile([P, 10, P], dt.bfloat16, tag="u1s",
                             name=f"u1s_{hc}")
                nc.vector.tensor_add(u1[:], t[:, 0:10], t[:, 10:20])
                u2 = up.tile([P, 5, P], dt.bfloat16, tag="u2s", name=f"u2s_{hc}")
                nc.vector.tensor_add(u2[:], u1[:, 0:5], u1[:, 5:10])
                u3 = up.tile([P, 2, P], dt.bfloat16, tag="u3s", name=f"u3s_{hc}")
                nc.vector.tensor_add(u3[:], u2[:, 0:2], u2[:, 2:4])
                u4 = up.tile([P, P], dt.bfloat16, tag="u4s", name=f"u4s_{hc}")
                nc.vector.tensor_add(u4[:], u3[:, 0], u3[:, 1])
                nc.vector.tensor_add(pall[:, hc, :], u4[:], u2[:, 4])
                nc.scalar.dma_start(piece[:, hc, :], pall[:, hc, :])
    _legalize_waits(nc)
    return nc


def build_gemm():
    """Launch 2: xh [4,128,8,256] bf16 @ wt [128,8,8,128] bf16 + RMSNorm."""
    nc = bass.Bass('TRN2', target_bir_lowering=False, debug=False)
    xh = nc.dram_tensor("xh", [NTG, P, NCH, TGW], dt.bfloat16, kind="ExternalInput")
    wt = nc.dram_tensor("wt", [P, NCH, NCORES, P], dt.bfloat16, kind="ExternalInput")
    nw = nc.dram_tensor("nw", [H], dt.bfloat16, kind="ExternalInput")
    y = nc.dram_tensor("y", [TPC, H], dt.bfloat16, kind="ExternalOutput")
    scratch = nc.dram_tensor("scratch", [P, H], dt.bfloat16, kind="Internal")
    mult = mybir.AluOpType.mult
    with TileContext(nc) as tc:
        with (
            tc.tile_pool(name="w", bufs=1) as wp,
            tc.tile_pool(name="sq", bufs=2) as sqp,
            tc.tile_pool(name="stat", bufs=8) as stat,
            tc.tile_pool(name="y", bufs=3) as yp,
            tc.tile_pool(name="psum", bufs=4, space="PSUM") as psum,
        ):
            nwb = wp.tile([P, H], dt.bfloat16, tag="nwb")
            wm = wp.tile([P, 512], dt.bfloat16, tag="wm")

            # PE warm-up while the first chunks stream in (cold PE = 1.2 GHz)
            nc.vector.memset(wm[:], 0.0)
            wu = psum.tile([P, H], dt.float32, tag="pt", name="wu")
            for i in range(11):
                nc.tensor.matmul(wu[:, 0:512], wm[:, 0:P], wm[:],
                                 start=True, stop=True)

            # separate SBUF tiles per chunk so matmuls depend only on the
            # chunk they read; single input ring (sync), priority order.
            # hc=0 is extra-fine (wt o-halves, per-tg hc0 x slices) so the
            # first matmuls fire while the DMA ring is still ramping up.
            wtc = [wp.tile([P, NCORES, P], dt.bfloat16, tag=f"wt{hc}",
                           name=f"wtc{hc}") for hc in range(NCH)]
            xhh = {(tg, hf): wp.tile([P, NCH // 2, TGW], dt.bfloat16,
                                     tag=f"xh{tg}_{hf}", name=f"xhh{tg}_{hf}")
                   for tg in range(NTG) for hf in range(2)}

            def dma_wt(hc):
                nc.sync.dma_start(wtc[hc][:], wt[:, hc])

            def dma_xh(tg, hf):
                nc.scalar.dma_start(xhh[tg, hf][:],
                                  xh[tg, :, hf * (NCH // 2):(hf + 1) * (NCH // 2)])

            dma_wt(0)
            dma_xh(0, 0)
            dma_xh(1, 0)
            dma_wt(1)
            dma_wt(2)
            dma_xh(0, 1)
            dma_xh(1, 1)
            dma_wt(3)
            dma_wt(4)
            dma_xh(2, 0)
            dma_xh(3, 0)
            dma_wt(5)
            dma_wt(6)
            dma_xh(2, 1)
            dma_xh(3, 1)
            dma_wt(7)
            nc.scalar.dma_start(nwb[:], nw[None, :].partition_broadcast(P))

            def norm_tile(tt, pt):
                # ssq on ACT (Square+accum, reads PSUM; Square/Sqrt share one
                # table set), then ACT Copy evicts PSUM to bf16; DVE only does
                # the tiny stats plus the 4x/2x scale ops
                sq = sqp.tile([P, H], dt.bfloat16, tag="sq", name=f"sq{tt}")
                vb = stat.tile([P, 1], dt.float32, tag="vb", name=f"vb{tt}")
                nc.scalar.activation(sq[:], pt[:],
                                     mybir.ActivationFunctionType.Square,
                                     accum_out=vb[:])
                yc = yp.tile([P, H], dt.bfloat16, tag="yc", name=f"yc{tt}")
                nc.scalar.activation(yc[:], pt[:],
                                     mybir.ActivationFunctionType.Copy)
                nc.vector.tensor_scalar(
                    vb[:], vb[:], SSQ_BIAS, None, mybir.AluOpType.add,
                )
                rv = stat.tile([P, 1], dt.float32, tag="rv", name=f"rv{tt}")
                nc.vector.reciprocal(rv[:], vb[:])
                s = stat.tile([P, 1], dt.float32, tag="s", name=f"s{tt}")
                nc.scalar.activation(
                    s[:], rv[:], mybir.ActivationFunctionType.Sqrt,
                    scale=float(H),
                )
                yt = yp.tile([P, H], dt.bfloat16, tag="yt", name=f"yt{tt}")
                nc.vector.tensor_scalar_mul(yt[:], yc[:], s[:])
                ysb = yp.tile([P, H], dt.bfloat16, tag="ysb", name=f"ysb{tt}")
                nc.vector.tensor_tensor(ysb[:], yt[:], nwb[:], mult)
                if tt in (3, 5, 6):
                    # contiguous dummy writes re-warm the idle output ring
                    # before/while the y stream runs
                    half = (tt - 3) % 2
                    nc.sync.dma_start(scratch[:, half * 512:(half + 1) * 512],
                                      yc[:, half * 512:(half + 1) * 512])
                nc.sync.dma_start(y[tt * P:(tt + 1) * P, :], ysb[:])

            def mm_tile(pt, tt, hc):
                tg, th = tt // 2, (tt % 2) * P
                lhsT = xhh[tg, hc // 4][:, hc % 4, th:th + P]
                for oh in range(2):
                    nc.tensor.matmul(
                        pt[:, oh * 512:(oh + 1) * 512],
                        lhsT,
                        wtc[hc][:, 4 * oh:4 * oh + 4, :],
                        start=(hc == 0), stop=(hc == NCH - 1),
                    )

            # group 0 (tiles 0-2): hc-outer — start on the first chunk
            pts0 = [psum.tile([P, H], dt.float32, tag="pt", name=f"pt{tt}")
                    for tt in range(3)]
            for hc in range(NCH):
                for tt in range(3):
                    mm_tile(pts0[tt], tt, hc)
            for tt in range(3):
                norm_tile(tt, pts0[tt])

            # group 1 (tiles 3-7): tile-outer — weights resident, staggered
            # stops; PSUM banks freed by group-0 norms well before reuse
            for tt in range(3, NT):
                pt = psum.tile([P, H], dt.float32, tag="pt", name=f"pt{tt}")
                for hc in range(NCH):
                    mm_tile(pt, tt, hc)
                norm_tile(tt, pt)
    _legalize_waits(nc)
    return nc


_CACHE = {}


def _get(name, builder):
    if name not in _CACHE:
        _CACHE[name] = builder()
    return _CACHE[name]


def make_wprep_inputs(conv_w):
    """[20,1024,1024] f32 -> per-core [8 hc,128 h,20 l,128 o] bf16."""
    bf16 = ml_dtypes.bfloat16
    conv_w = np.asarray(conv_w, dtype=np.float32)
    in_maps = []
    for c in range(NCORES):
        a = conv_w[:, c * P:(c + 1) * P, :]          # [20 l, 128 o, 1024 h]
        r = a.reshape(NL, P, NCH, P)                 # [l, o, hc, h]
        cwc = r.transpose(2, 3, 0, 1)                # [hc, h, l, o]
        in_maps.append({"cw": np.ascontiguousarray(cwc.astype(bf16))})
    return in_maps


def assemble_wt(pieces):
    """8 x [128 h,8 hc,128 o_c] -> [128 p(h),8 hc,8 c,128 o] bf16."""
    stacked = np.stack(pieces, axis=2)               # [h, hc, c, o]
    return np.ascontiguousarray(stacked.astype(ml_dtypes.bfloat16))


def make_gemm_inputs(hidden_states, wt_host, norm_w):
    bf16 = ml_dtypes.bfloat16
    x = np.asarray(hidden_states, dtype=np.float32).reshape(TOK, H)
    nw = np.ascontiguousarray(np.asarray(norm_w, dtype=np.float32).astype(bf16))
    in_maps = []
    for c in range(NCORES):
        xc = x[c * TPC:(c + 1) * TPC]                # [1024 t, 1024 h]
        xhc = np.ascontiguousarray(
            xc.reshape(NTG, TGW, NCH, P).transpose(0, 3, 2, 1).astype(bf16)
        )
        in_maps.append({"xh": xhc, "wt": wt_host, "nw": nw})
    return in_maps


def kernel(hidden_states, conv_w, norm_w):
    in_dtype = np.asarray(hidden_states).dtype
    core_ids = list(range(NCORES))

    nc1 = _get("wprep", build_wprep)
    res1 = bass_utils.run_bass_kernel_spmd(nc1, make_wprep_inputs(conv_w), core_ids)
    wt_host = assemble_wt([res1.results[i]["piece"] for i in range(NCORES)])

    nc2 = _get("gemm", build_gemm)
    res2 = bass_utils.run_bass_kernel_spmd(
        nc2, make_gemm_inputs(hidden_states, wt_host, norm_w), core_ids)
    ys = [res2.results[i]["y"].astype(np.float32) for i in range(NCORES)]
    return np.concatenate(ys, axis=0).reshape(B, S, H).astype(in_dtype, copy=False)
